# revision 14
# baseline (speedup 1.0000x reference)
"""Int8-dequant linear layer (out = input @ (qweight*scale).T + bias) on 8 trn2 cores.

Sharding: token-parallel. input [8,512,4096] flattens to 4096 tokens; each
core computes 512 tokens against the full weight matrix.

Structure (v3):
  * x ships from the host already transposed+fp16 ([P, TT, KT, P] layout), so
    the Tensor engine does nothing but matmuls (no PE transposes).
  * of-chunks are processed in PAIRS: the two matmuls of a pair share one
    stationary x-tile. The tile lowering splits every matmul into
    InstLdweights + InstMatmult(ldweights=False); a post-pass removes the
    second, identical InstLdweights of each pair (the PE array still holds
    the operand), halving PE weight-load events.
  * PSUM: 3 accumulation rounds x 2 banks + 1 warm-up bank.
  * prologue DMA order: first token-tile of x first (gates the first matmul),
    bias (first needed ~60us in) last; ~20 zero matmuls keep the PE (and its
    HAM clock-gate) warm during the initial DMA gate.

qweight ships as int8 in [of_chunk, partition, k_tile, n] layout and is cast
to fp16 by the SWDGE DMA (exact for ints in [-127,127]). Scale and bias are
applied in fp32 after PSUM accumulation (ScalarE mul + VectorE add), so the
only precision loss vs the fp32 reference is fp16 rounding of the
activations (~2e-4 relative).
"""

import numpy as np

B, S, IN_F, OUT_F = 8, 512, 4096, 4096
N_CORES = 8
TOK = B * S                # 4096 tokens total
TOK_C = TOK // N_CORES     # 512 tokens per core
P = 128                    # partitions
KT = IN_F // P             # 32 k-tiles
NT = 512                   # out-feature chunk (one fp32 PSUM bank)
OF_CHUNKS = OUT_F // NT    # 8
TT = TOK_C // P            # 4 token tiles per core
PAIRS = OF_CHUNKS // 2     # 4

DEDUPE_LDW = True          # drop the pair's second identical InstLdweights
FULL_REP = False           # bench mode: repeat prologue DMAs inside each rep
IOBUFS = 1                 # bench mode: iopool bufs (2 decouples FULL_REP reps)


def _make_tile_context_cls():
    import bass_rust
    import concourse.mybir as mybir
    from concourse.tile import TileContext, ScopedClock

    class _TC(TileContext):
        # The walrus build in this image rejects more than one semaphore wait
        # per instruction. Split extra waits onto nofuse NOPs committed just
        # before the instruction on the same engine (identical queue
        # semantics: the sequencer blocks on the NOP's wait first).
        def _commit_instruction(self, inst, lazy_reg_writes: bool = True):
            si = getattr(inst, "sync_info", None)
            if (
                si is not None
                and len(si.on_wait) > 1
                and inst.engine != mybir.EngineType.Unassigned
            ):
                waits = list(si.on_wait)
                for i, w in enumerate(waits[:-1]):
                    nop = mybir.InstNoOp(
                        name=f"{inst.name}-ws{i}",
                        sync_info=mybir.SyncInfo(on_wait=[w], on_update=[]),
                        bass_nofuse=True,
                        engine=inst.engine,
                    )
                    self._add_instruction(nop)
                inst.sync_info = mybir.SyncInfo(
                    on_wait=[waits[-1]], on_update=list(si.on_update)
                )
            return super()._commit_instruction(inst, lazy_reg_writes)

        # Same walrus limitation: it can't encode syncs on the exit Drain, so
        # land the end-of-kernel clock waits on single-wait NOPs and use the
        # sequencer-level (EVSEM-only) barrier instead of the drain butterfly.
        def _drain_and_barrier(self, tick_clock, wait_clock):
            nc = self.nc
            carrier = nc.sync.nop(nofuse=True)
            wait_clock.add_sem_waits(
                carrier.ins, ScopedClock({None: tick_clock.global_clock})
            )
            waits = list(carrier.ins.sync_info.on_wait)
            if len(waits) > 1:
                carrier.ins.sync_info = bass_rust.SyncInfo(
                    on_wait=[waits[0]], on_update=[]
                )
                for w in waits[1:]:
                    extra = nc.sync.nop(nofuse=True)
                    extra.ins.sync_info = bass_rust.SyncInfo(
                        on_wait=[w], on_update=[]
                    )
            nc.sync.drain()
            nc.all_engine_barrier(sem_only=True)
            assert self.sems is not None
            popped = nc._tile_sem_poison_stack.pop()
            assert popped is self._sem_poison
            nc.clear_and_free_semaphores(list(self.sems.allocated().values()))
            nc.all_engine_barrier(sem_only=True)

    return _TC


def _dedupe_ldweights(nc):
    """Remove consecutive identical InstLdweights (the PE array already
    holds the operand). Conservative: only drops sync-free duplicates with
    nothing but non-self-loading matmuls in between."""
    import concourse.mybir as mybir

    for f in nc.m.functions:
        for bb in f.blocks:
            keep = []
            last_key = None
            for ins in bb.instructions:
                if isinstance(ins, mybir.InstLdweights):
                    key = str(ins.ins[0])
                    si = ins.sync_info
                    empty = (si is None) or (
                        not si.on_wait and not si.on_update
                    )
                    if key == last_key and empty:
                        continue
                    last_key = key
                elif isinstance(ins, mybir.InstMatmult):
                    pass  # non-self-loading: stationary state preserved
                else:
                    last_key = None
                keep.append(ins)
            bb.instructions = keep


def build_nc(reps=1):
    """Build the per-core Bass program (SPMD: same program, different x shard).

    reps>1 repeats the compute body (same inputs/outputs) for benchmarking:
    (T(reps=R) - T(reps=1)) / (R-1) cancels dispatch overhead. With
    FULL_REP the prologue DMAs repeat too (per-rep ~ single-pass span).
    """
    import concourse.bass as bass
    import concourse.mybir as mybir

    f16 = mybir.dt.float16
    f32 = mybir.dt.float32

    nc = bass.Bass("TRN2", target_bir_lowering=False, debug=False)
    # x ships pre-transposed fp16: xt[p, t, j, c] = x[t*128+c, j*128+p]
    xt_d = nc.dram_tensor("xt", [P, TT, KT, P], f16, kind="ExternalInput").ap()
    # weights ship as int8 (exact) and are cast to fp16 inline by the SWDGE
    # DMA - halves weight HBM traffic vs fp16-in-DRAM.
    wt = nc.dram_tensor(
        "wt", [OF_CHUNKS, P, KT, NT], mybir.dt.int8, kind="ExternalInput"
    ).ap()
    # bias comes pre-broadcast to 128 partitions from the host: a plain
    # contiguous 2MB DMA beats a [1,N]->[128,N] broadcast DMA.
    bias = nc.dram_tensor("bias", [P, OUT_F], f32, kind="ExternalInput").ap()
    scale = nc.dram_tensor("scale", [1, 1], f32, kind="ExternalInput").ap()
    out = nc.dram_tensor("out", [TOK_C, OUT_F], f32, kind="ExternalOutput").ap()

    TC = _make_tile_context_cls()
    with TC(nc) as tc:
        with (
            tc.tile_pool(name="iopool", bufs=IOBUFS) as iopool,
            tc.tile_pool(name="wpool", bufs=2) as wpool,
            tc.tile_pool(name="opool", bufs=4) as opool,
            tc.tile_pool(name="pacc", bufs=3, space="PSUM") as pacc_pool,
            tc.tile_pool(name="pwarm", bufs=1, space="PSUM") as pwarm_pool,
        ):
            def prologue():
                # xt[0] gates the first matmul: issue it first on the sync
                # queue (front half first). bias is first consumed ~60us in:
                # issue it last.
                xt = iopool.tile([P, TT, KT, P], f16)
                nc.sync.dma_start(out=xt[:, 0, 0:KT // 2], in_=xt_d[:, 0, 0:KT // 2])
                nc.sync.dma_start(out=xt[:, 0, KT // 2:], in_=xt_d[:, 0, KT // 2:])
                scale_sb = iopool.tile([P, 1], f32)
                nc.sync.dma_start(
                    out=scale_sb, in_=scale.to_broadcast((P, 1))
                )
                for t in range(1, TT):
                    nc.sync.dma_start(out=xt[:, t], in_=xt_d[:, t])
                bias_sb = iopool.tile([P, OUT_F], f32)
                nc.sync.dma_start(out=bias_sb, in_=bias)
                return xt, scale_sb, bias_sb

            def warmup():
                # Keep the PE busy (and the HAM clock-gate open) during the
                # initial DMA gate: ~20 zero matmuls into a scratch PSUM
                # tile. The dedupe pass collapses their ldweights to one.
                warm = iopool.tile([P, NT], f16)
                nc.vector.memset(warm, 0.0)
                wacc = pwarm_pool.tile([P, NT], f32)
                for i in range(20):
                    nc.tensor.matmul(
                        wacc, warm[:, 0:P], warm,
                        start=True, stop=True,
                    )

            if FULL_REP:
                for rep in range(reps):
                    xt, scale_sb, bias_sb = prologue()
                    if rep == 0:
                        warmup()
                    _body(nc, wpool, opool, pacc_pool, wt, out, xt,
                          scale_sb, bias_sb)
            else:
                xt, scale_sb, bias_sb = prologue()
                warmup()
                for rep in range(reps):
                    _body(nc, wpool, opool, pacc_pool, wt, out, xt,
                          scale_sb, bias_sb)
    if DEDUPE_LDW:
        _dedupe_ldweights(nc)
    return nc


def _body(nc, wpool, opool, pacc_pool, wt, out, xt, scale_sb, bias_sb):
    import concourse.mybir as mybir
    f16 = mybir.dt.float16
    f32 = mybir.dt.float32
    for pair in range(PAIRS):
        of0, of1 = 2 * pair, 2 * pair + 1
        wc0 = wpool.tile([P, KT, NT], f16)
        wc1 = wpool.tile([P, KT, NT], f16)
        if pair == 0:
            # split the first chunk loads so matmul j can start once its
            # k-block is resident
            for q in range(8):
                nc.gpsimd.dma_start(
                    out=wc0[:, q * (KT // 8):(q + 1) * (KT // 8), :],
                    in_=wt[of0, :, q * (KT // 8):(q + 1) * (KT // 8), :],
                )
            for q in range(2):
                nc.gpsimd.dma_start(
                    out=wc1[:, q * (KT // 2):(q + 1) * (KT // 2), :],
                    in_=wt[of1, :, q * (KT // 2):(q + 1) * (KT // 2), :],
                )
        else:
            nc.gpsimd.dma_start(out=wc0, in_=wt[of0])  # int8 -> fp16 cast
            nc.gpsimd.dma_start(out=wc1, in_=wt[of1])
        for t in range(TT):
            acc = pacc_pool.tile([P, 2, NT], f32)  # 2 PSUM banks
            for j in range(KT):
                lhsT = xt[:, t, j, :]
                first, last = (j == 0), (j == KT - 1)
                nc.tensor.matmul(
                    acc[:, 0, :], lhsT, wc0[:, j, :], start=first, stop=last
                )
                nc.tensor.matmul(
                    acc[:, 1, :], lhsT, wc1[:, j, :], start=first, stop=last
                )
            osb = opool.tile([P, 2, NT], f32)
            nc.scalar.mul(osb, acc, scale_sb[:, :])
            nc.vector.tensor_add(
                osb, osb, bias_sb[:, of0 * NT:(of1 + 1) * NT]
            )
            nc.sync.dma_start(
                out=out[t * P:(t + 1) * P, of0 * NT:(of1 + 1) * NT],
                in_=osb,
            )


def prep_inputs(input, qweight, weight_scale, bias_param):
    """Host-side shard/repack. Returns per-core in_maps."""
    X = np.asarray(input, dtype=np.float32).reshape(TOK, IN_F)
    # int8 container for the int8-valued weights; the device DMA casts to fp16
    # (exact for integers in [-127,127]).
    q8 = np.asarray(qweight).astype(np.int8)
    # w_packed[of, p, j, n] = qweight[of*NT + n, j*P + p]
    wp = np.ascontiguousarray(
        q8.reshape(OF_CHUNKS, NT, KT, P).transpose(0, 3, 2, 1)
    )
    bias2 = np.ascontiguousarray(
        np.broadcast_to(
            np.asarray(bias_param, dtype=np.float32).reshape(1, OUT_F), (P, OUT_F)
        )
    )
    scale2 = np.ascontiguousarray(
        np.asarray(weight_scale, dtype=np.float32).reshape(1, 1)
    )
    in_maps = []
    for c in range(N_CORES):
        Xc = X[c * TOK_C:(c + 1) * TOK_C].astype(np.float16)
        # xt[p, t, j, c] = x[t*128+c, j*128+p]
        xt = np.ascontiguousarray(
            Xc.reshape(TT, P, KT, P).transpose(3, 0, 2, 1)
        )
        in_maps.append(
            {
                "xt": xt,
                "wt": wp,
                "bias": bias2,
                "scale": scale2,
            }
        )
    return in_maps


def assemble_output(results):
    out = np.concatenate([results[c]["out"] for c in range(N_CORES)], axis=0)
    return np.ascontiguousarray(out.reshape(B, S, OUT_F).astype(np.float32))


def kernel(input, qweight, weight_scale, bias_param):
    from concourse.bass_utils import run_bass_kernel_spmd

    in_maps = prep_inputs(input, qweight, weight_scale, bias_param)
    nc = build_nc()
    res = run_bass_kernel_spmd(nc, in_maps, core_ids=list(range(N_CORES)))
    return assemble_output(res.results)


# revision 17
# speedup vs baseline: 1.0563x; 1.0563x over previous
"""Int8-dequant linear layer (out = input @ (qweight*scale).T + bias) on 8 trn2 cores.

Sharding: token-parallel. input [8,512,4096] flattens to 4096 tokens; each
core computes 512 tokens against the full weight matrix.

Structure (v3):
  * x ships from the host already transposed+fp16 ([P, TT, KT, P] layout), so
    the Tensor engine does nothing but matmuls (no PE transposes).
  * of-chunks are processed in PAIRS: the two matmuls of a pair share one
    stationary x-tile. The tile lowering splits every matmul into
    InstLdweights + InstMatmult(ldweights=False); a post-pass removes the
    second, identical InstLdweights of each pair (the PE array still holds
    the operand), halving PE weight-load events.
  * pacc pool owns all 8 PSUM banks (4 bufs x 2 banks).
  * prologue DMA order: first token-tile of x first (gates the first matmul),
    bias (first needed ~60us in) last; ~20 zero matmuls keep the PE (and its
    HAM clock-gate) warm during the initial DMA gate.

qweight ships as int8 in [of_chunk, partition, k_tile, n] layout and is cast
to fp16 by the SWDGE DMA (exact for ints in [-127,127]). Scale and bias are
applied in fp32 after PSUM accumulation (ScalarE mul + VectorE add), so the
only precision loss vs the fp32 reference is fp16 rounding of the
activations (~2e-4 relative).
"""

import numpy as np

B, S, IN_F, OUT_F = 8, 512, 4096, 4096
N_CORES = 8
TOK = B * S                # 4096 tokens total
TOK_C = TOK // N_CORES     # 512 tokens per core
P = 128                    # partitions
KT = IN_F // P             # 32 k-tiles
NT = 512                   # out-feature chunk (one fp32 PSUM bank)
OF_CHUNKS = OUT_F // NT    # 8
TT = TOK_C // P            # 4 token tiles per core
PAIRS = OF_CHUNKS // 2     # 4

DEDUPE_LDW = True          # drop the pair's second identical InstLdweights
FULL_REP = False           # bench mode: repeat prologue DMAs inside each rep
IOBUFS = 1                 # bench mode: iopool bufs (2 decouples FULL_REP reps)


def _make_tile_context_cls():
    import bass_rust
    import concourse.mybir as mybir
    from concourse.tile import TileContext, ScopedClock

    class _TC(TileContext):
        # The walrus build in this image rejects more than one semaphore wait
        # per instruction. Split extra waits onto nofuse NOPs committed just
        # before the instruction on the same engine (identical queue
        # semantics: the sequencer blocks on the NOP's wait first).
        def _commit_instruction(self, inst, lazy_reg_writes: bool = True):
            si = getattr(inst, "sync_info", None)
            if (
                si is not None
                and len(si.on_wait) > 1
                and inst.engine != mybir.EngineType.Unassigned
            ):
                waits = list(si.on_wait)
                for i, w in enumerate(waits[:-1]):
                    nop = mybir.InstNoOp(
                        name=f"{inst.name}-ws{i}",
                        sync_info=mybir.SyncInfo(on_wait=[w], on_update=[]),
                        bass_nofuse=True,
                        engine=inst.engine,
                    )
                    self._add_instruction(nop)
                inst.sync_info = mybir.SyncInfo(
                    on_wait=[waits[-1]], on_update=list(si.on_update)
                )
            return super()._commit_instruction(inst, lazy_reg_writes)

        # Same walrus limitation: it can't encode syncs on the exit Drain, so
        # land the end-of-kernel clock waits on single-wait NOPs and use the
        # sequencer-level (EVSEM-only) barrier instead of the drain butterfly.
        def _drain_and_barrier(self, tick_clock, wait_clock):
            nc = self.nc
            carrier = nc.sync.nop(nofuse=True)
            wait_clock.add_sem_waits(
                carrier.ins, ScopedClock({None: tick_clock.global_clock})
            )
            waits = list(carrier.ins.sync_info.on_wait)
            if len(waits) > 1:
                carrier.ins.sync_info = bass_rust.SyncInfo(
                    on_wait=[waits[0]], on_update=[]
                )
                for w in waits[1:]:
                    extra = nc.sync.nop(nofuse=True)
                    extra.ins.sync_info = bass_rust.SyncInfo(
                        on_wait=[w], on_update=[]
                    )
            nc.sync.drain()
            nc.all_engine_barrier(sem_only=True)
            assert self.sems is not None
            popped = nc._tile_sem_poison_stack.pop()
            assert popped is self._sem_poison
            nc.clear_and_free_semaphores(list(self.sems.allocated().values()))
            nc.all_engine_barrier(sem_only=True)

    return _TC


def _dedupe_ldweights(nc):
    """Remove consecutive identical InstLdweights (the PE array already
    holds the operand). Conservative: only drops sync-free duplicates with
    nothing but non-self-loading matmuls in between."""
    import concourse.mybir as mybir

    for f in nc.m.functions:
        for bb in f.blocks:
            keep = []
            last_key = None
            for ins in bb.instructions:
                if isinstance(ins, mybir.InstLdweights):
                    key = str(ins.ins[0])
                    si = ins.sync_info
                    empty = (si is None) or (
                        not si.on_wait and not si.on_update
                    )
                    if key == last_key and empty:
                        continue
                    last_key = key
                elif isinstance(ins, mybir.InstMatmult):
                    pass  # non-self-loading: stationary state preserved
                else:
                    last_key = None
                keep.append(ins)
            bb.instructions = keep


def build_nc(reps=1):
    """Build the per-core Bass program (SPMD: same program, different x shard).

    reps>1 repeats the compute body (same inputs/outputs) for benchmarking:
    (T(reps=R) - T(reps=1)) / (R-1) cancels dispatch overhead. With
    FULL_REP the prologue DMAs repeat too (per-rep ~ single-pass span).
    """
    import concourse.bass as bass
    import concourse.mybir as mybir

    f16 = mybir.dt.float16
    f32 = mybir.dt.float32

    nc = bass.Bass("TRN2", target_bir_lowering=False, debug=False)
    # x ships pre-transposed fp16: xt[p, t, j, c] = x[t*128+c, j*128+p]
    xt_d = nc.dram_tensor("xt", [P, TT, KT, P], f16, kind="ExternalInput").ap()
    # weights ship as int8 (exact) and are cast to fp16 inline by the SWDGE
    # DMA - halves weight HBM traffic vs fp16-in-DRAM.
    wt = nc.dram_tensor(
        "wt", [OF_CHUNKS, P, KT, NT], mybir.dt.int8, kind="ExternalInput"
    ).ap()
    # bias comes pre-broadcast to 128 partitions from the host: a plain
    # contiguous 2MB DMA beats a [1,N]->[128,N] broadcast DMA.
    bias = nc.dram_tensor("bias", [P, OUT_F], f32, kind="ExternalInput").ap()
    scale = nc.dram_tensor("scale", [1, 1], f32, kind="ExternalInput").ap()
    out = nc.dram_tensor("out", [TOK_C, OUT_F], f32, kind="ExternalOutput").ap()

    TC = _make_tile_context_cls()
    with TC(nc) as tc:
        with (
            tc.tile_pool(name="iopool", bufs=IOBUFS) as iopool,
            tc.tile_pool(name="wpool", bufs=2) as wpool,
            tc.tile_pool(name="opool", bufs=4) as opool,
            tc.tile_pool(name="pacc", bufs=4, space="PSUM") as pacc_pool,
        ):
            def prologue():
                # xt[0] gates the first matmul: issue it first on the sync
                # queue (front half first). bias is first consumed ~60us in:
                # issue it last.
                xt = iopool.tile([P, TT, KT, P], f16)
                nc.sync.dma_start(out=xt[:, 0, 0:KT // 2], in_=xt_d[:, 0, 0:KT // 2])
                nc.sync.dma_start(out=xt[:, 0, KT // 2:], in_=xt_d[:, 0, KT // 2:])
                scale_sb = iopool.tile([P, 1], f32)
                nc.sync.dma_start(
                    out=scale_sb, in_=scale.to_broadcast((P, 1))
                )
                for t in range(1, TT):
                    nc.sync.dma_start(out=xt[:, t], in_=xt_d[:, t])
                bias_sb = iopool.tile([P, OUT_F], f32)
                nc.sync.dma_start(out=bias_sb, in_=bias)
                return xt, scale_sb, bias_sb

            def warmup():
                # Zero tile for PE warm-up matmuls (issued by _body into the
                # first accumulation tile, before its real j-loop).
                warm = iopool.tile([P, NT], f16)
                nc.vector.memset(warm, 0.0)
                return warm

            if FULL_REP:
                for rep in range(reps):
                    xt, scale_sb, bias_sb = prologue()
                    warm = warmup() if rep == 0 else None
                    _body(nc, wpool, opool, pacc_pool, wt, out, xt,
                          scale_sb, bias_sb, warm)
            else:
                xt, scale_sb, bias_sb = prologue()
                warm = warmup()
                for rep in range(reps):
                    _body(nc, wpool, opool, pacc_pool, wt, out, xt,
                          scale_sb, bias_sb, warm)
                    warm = None
    if DEDUPE_LDW:
        _dedupe_ldweights(nc)
    return nc


def _body(nc, wpool, opool, pacc_pool, wt, out, xt, scale_sb, bias_sb,
          warm=None):
    import concourse.mybir as mybir
    f16 = mybir.dt.float16
    f32 = mybir.dt.float32
    for pair in range(PAIRS):
        of0, of1 = 2 * pair, 2 * pair + 1
        wc0 = wpool.tile([P, KT, NT], f16)
        wc1 = wpool.tile([P, KT, NT], f16)
        if pair == 0:
            # split the first chunk loads so matmul j can start once its
            # k-block is resident
            for q in range(8):
                nc.gpsimd.dma_start(
                    out=wc0[:, q * (KT // 8):(q + 1) * (KT // 8), :],
                    in_=wt[of0, :, q * (KT // 8):(q + 1) * (KT // 8), :],
                )
            for q in range(2):
                nc.gpsimd.dma_start(
                    out=wc1[:, q * (KT // 2):(q + 1) * (KT // 2), :],
                    in_=wt[of1, :, q * (KT // 2):(q + 1) * (KT // 2), :],
                )
        else:
            nc.gpsimd.dma_start(out=wc0, in_=wt[of0])  # int8 -> fp16 cast
            nc.gpsimd.dma_start(out=wc1, in_=wt[of1])
        for t in range(TT):
            acc = pacc_pool.tile([P, 2, NT], f32)  # 2 PSUM banks
            if warm is not None and pair == 0 and t == 0:
                # ~20 zero matmuls with no DMA dependency: keeps the PE busy
                # (HAM clock-gate open) during the initial DMA gate. The
                # real j=0 matmul below re-clears the bank (start=True).
                for _ in range(20):
                    nc.tensor.matmul(
                        acc[:, 0, :], warm[:, 0:P], warm,
                        start=True, stop=True,
                    )
            for j in range(KT):
                lhsT = xt[:, t, j, :]
                first, last = (j == 0), (j == KT - 1)
                nc.tensor.matmul(
                    acc[:, 0, :], lhsT, wc0[:, j, :], start=first, stop=last
                )
                nc.tensor.matmul(
                    acc[:, 1, :], lhsT, wc1[:, j, :], start=first, stop=last
                )
            osb = opool.tile([P, 2, NT], f32)
            nc.scalar.mul(osb, acc, scale_sb[:, :])
            nc.vector.tensor_add(
                osb, osb, bias_sb[:, of0 * NT:(of1 + 1) * NT]
            )
            nc.sync.dma_start(
                out=out[t * P:(t + 1) * P, of0 * NT:(of1 + 1) * NT],
                in_=osb,
            )


def prep_inputs(input, qweight, weight_scale, bias_param):
    """Host-side shard/repack. Returns per-core in_maps."""
    X = np.asarray(input, dtype=np.float32).reshape(TOK, IN_F)
    # int8 container for the int8-valued weights; the device DMA casts to fp16
    # (exact for integers in [-127,127]).
    q8 = np.asarray(qweight).astype(np.int8)
    # w_packed[of, p, j, n] = qweight[of*NT + n, j*P + p]
    wp = np.ascontiguousarray(
        q8.reshape(OF_CHUNKS, NT, KT, P).transpose(0, 3, 2, 1)
    )
    bias2 = np.ascontiguousarray(
        np.broadcast_to(
            np.asarray(bias_param, dtype=np.float32).reshape(1, OUT_F), (P, OUT_F)
        )
    )
    scale2 = np.ascontiguousarray(
        np.asarray(weight_scale, dtype=np.float32).reshape(1, 1)
    )
    in_maps = []
    for c in range(N_CORES):
        Xc = X[c * TOK_C:(c + 1) * TOK_C].astype(np.float16)
        # xt[p, t, j, c] = x[t*128+c, j*128+p]
        xt = np.ascontiguousarray(
            Xc.reshape(TT, P, KT, P).transpose(3, 0, 2, 1)
        )
        in_maps.append(
            {
                "xt": xt,
                "wt": wp,
                "bias": bias2,
                "scale": scale2,
            }
        )
    return in_maps


def assemble_output(results):
    out = np.concatenate([results[c]["out"] for c in range(N_CORES)], axis=0)
    return np.ascontiguousarray(out.reshape(B, S, OUT_F).astype(np.float32))


def kernel(input, qweight, weight_scale, bias_param):
    from concourse.bass_utils import run_bass_kernel_spmd

    in_maps = prep_inputs(input, qweight, weight_scale, bias_param)
    nc = build_nc()
    res = run_bass_kernel_spmd(nc, in_maps, core_ids=list(range(N_CORES)))
    return assemble_output(res.results)


# revision 23
# speedup vs baseline: 1.1351x; 1.0746x over previous
"""Int8-dequant linear layer (out = input @ (qweight*scale).T + bias) on 8 trn2 cores.

Sharding: token-parallel. input [8,512,4096] flattens to 4096 tokens; each
core computes 512 tokens against the full weight matrix.

Structure (v3):
  * x ships from the host already transposed+fp16 ([P, TT, KT, P] layout), so
    the Tensor engine does nothing but matmuls (no PE transposes).
  * of-chunks are processed in PAIRS: the two matmuls of a pair share one
    stationary x-tile. The tile lowering splits every matmul into
    InstLdweights + InstMatmult(ldweights=False); a post-pass removes the
    second, identical InstLdweights of each pair (the PE array still holds
    the operand), halving PE weight-load events.
  * pacc pool owns all 8 PSUM banks (4 bufs x 2 banks).
  * prologue DMA order: first token-tile of x first (gates the first matmul),
    bias (first needed ~60us in) last; ~20 zero matmuls keep the PE (and its
    HAM clock-gate) warm during the initial DMA gate.

qweight ships as int8 in [of_chunk, partition, k_tile, n] layout and is cast
to fp16 by the SWDGE DMA (exact for ints in [-127,127]). Scale and bias are
applied in fp32 after PSUM accumulation (ScalarE mul + VectorE add), so the
only precision loss vs the fp32 reference is fp16 rounding of the
activations (~2e-4 relative).
"""

import numpy as np

B, S, IN_F, OUT_F = 8, 512, 4096, 4096
N_CORES = 8
TOK = B * S                # 4096 tokens total
TOK_C = TOK // N_CORES     # 512 tokens per core
P = 128                    # partitions
KT = IN_F // P             # 32 k-tiles
NT = 512                   # out-feature chunk (one fp32 PSUM bank)
OF_CHUNKS = OUT_F // NT    # 8
TT = TOK_C // P            # 4 token tiles per core
PAIRS = OF_CHUNKS // 2     # 4

DEDUPE_LDW = True          # drop the pair's second identical InstLdweights
FULL_REP = False           # bench mode: repeat prologue DMAs inside each rep
IOBUFS = 1                 # bench mode: iopool bufs (2 decouples FULL_REP reps)


def _make_tile_context_cls():
    import bass_rust
    import concourse.mybir as mybir
    from concourse.tile import TileContext, ScopedClock

    class _TC(TileContext):
        # The walrus build in this image rejects more than one semaphore wait
        # per instruction. Split extra waits onto nofuse NOPs committed just
        # before the instruction on the same engine (identical queue
        # semantics: the sequencer blocks on the NOP's wait first).
        def _commit_instruction(self, inst, lazy_reg_writes: bool = True):
            si = getattr(inst, "sync_info", None)
            if (
                si is not None
                and len(si.on_wait) > 1
                and inst.engine != mybir.EngineType.Unassigned
            ):
                waits = list(si.on_wait)
                for i, w in enumerate(waits[:-1]):
                    nop = mybir.InstNoOp(
                        name=f"{inst.name}-ws{i}",
                        sync_info=mybir.SyncInfo(on_wait=[w], on_update=[]),
                        bass_nofuse=True,
                        engine=inst.engine,
                    )
                    self._add_instruction(nop)
                inst.sync_info = mybir.SyncInfo(
                    on_wait=[waits[-1]], on_update=list(si.on_update)
                )
            return super()._commit_instruction(inst, lazy_reg_writes)

        # Same walrus limitation: it can't encode syncs on the exit Drain, so
        # land the end-of-kernel clock waits on single-wait NOPs and use the
        # sequencer-level (EVSEM-only) barrier instead of the drain butterfly.
        def _drain_and_barrier(self, tick_clock, wait_clock):
            nc = self.nc
            carrier = nc.sync.nop(nofuse=True)
            wait_clock.add_sem_waits(
                carrier.ins, ScopedClock({None: tick_clock.global_clock})
            )
            waits = list(carrier.ins.sync_info.on_wait)
            if len(waits) > 1:
                carrier.ins.sync_info = bass_rust.SyncInfo(
                    on_wait=[waits[0]], on_update=[]
                )
                for w in waits[1:]:
                    extra = nc.sync.nop(nofuse=True)
                    extra.ins.sync_info = bass_rust.SyncInfo(
                        on_wait=[w], on_update=[]
                    )
            nc.sync.drain()
            nc.all_engine_barrier(sem_only=True)
            assert self.sems is not None
            popped = nc._tile_sem_poison_stack.pop()
            assert popped is self._sem_poison
            nc.clear_and_free_semaphores(list(self.sems.allocated().values()))
            nc.all_engine_barrier(sem_only=True)

    return _TC


def _dedupe_ldweights(nc):
    """Remove consecutive identical InstLdweights (the PE array already
    holds the operand). Conservative: only drops sync-free duplicates with
    nothing but non-self-loading matmuls in between."""
    import concourse.mybir as mybir

    for f in nc.m.functions:
        for bb in f.blocks:
            keep = []
            last_key = None
            for ins in bb.instructions:
                if isinstance(ins, mybir.InstLdweights):
                    key = str(ins.ins[0])
                    si = ins.sync_info
                    empty = (si is None) or (
                        not si.on_wait and not si.on_update
                    )
                    if key == last_key and empty:
                        continue
                    last_key = key
                elif isinstance(ins, mybir.InstMatmult):
                    pass  # non-self-loading: stationary state preserved
                else:
                    last_key = None
                keep.append(ins)
            bb.instructions = keep


def build_nc(reps=1):
    """Build the per-core Bass program (SPMD: same program, different x shard).

    reps>1 repeats the compute body (same inputs/outputs) for benchmarking:
    (T(reps=R) - T(reps=1)) / (R-1) cancels dispatch overhead. With
    FULL_REP the prologue DMAs repeat too (per-rep ~ single-pass span).
    """
    import concourse.bass as bass
    import concourse.mybir as mybir

    f16 = mybir.dt.float16
    f32 = mybir.dt.float32

    nc = bass.Bass("TRN2", target_bir_lowering=False, debug=False)
    # x ships pre-transposed fp16: xt[p, t, j, c] = x[t*128+c, j*128+p]
    xt_d = nc.dram_tensor("xt", [P, TT, KT, P], f16, kind="ExternalInput").ap()
    # weights ship as int8 (exact) and are cast to fp16 inline by the SWDGE
    # DMA - halves weight HBM traffic vs fp16-in-DRAM.
    wt = nc.dram_tensor(
        "wt", [OF_CHUNKS, P, KT, NT], mybir.dt.int8, kind="ExternalInput"
    ).ap()
    # pair 0's weights also ship pre-cast to fp16: they are needed in the
    # first ~14us, and the int8->fp16 cast-DMA path (~210GB/s into SBUF)
    # can't keep up with the PE at kernel start. The fp16 copy rides the
    # sync queue in parallel with the gpsimd cast queue.
    wt0 = nc.dram_tensor("wt0", [2, P, KT, NT], f16, kind="ExternalInput").ap()
    # bias comes pre-broadcast to 128 partitions from the host: a plain
    # contiguous 2MB DMA beats a [1,N]->[128,N] broadcast DMA.
    bias = nc.dram_tensor("bias", [P, OUT_F], f32, kind="ExternalInput").ap()
    scale = nc.dram_tensor("scale", [1, 1], f32, kind="ExternalInput").ap()
    out = nc.dram_tensor("out", [TOK_C, OUT_F], f32, kind="ExternalOutput").ap()

    TC = _make_tile_context_cls()
    with TC(nc) as tc:
        with (
            tc.tile_pool(name="iopool", bufs=IOBUFS) as iopool,
            tc.tile_pool(name="wpool", bufs=2) as wpool,
            tc.tile_pool(name="opool", bufs=4) as opool,
            tc.tile_pool(name="pacc", bufs=4, space="PSUM") as pacc_pool,
        ):
            def prologue():
                # Sync-queue issue order = delivery order. xt[0] gates the
                # first matmul; then pair-0 fp16 weights in front-loaded
                # pieces interleaved wc0/wc1 (the j loop consumes both in
                # lockstep); xt[1..3] next (first needed ~25us in); bias
                # (needed ~60us in) last.
                xt = iopool.tile([P, TT, KT, P], f16)
                nc.sync.dma_start(out=xt[:, 0, 0:KT // 2], in_=xt_d[:, 0, 0:KT // 2])
                nc.sync.dma_start(out=xt[:, 0, KT // 2:], in_=xt_d[:, 0, KT // 2:])
                scale_sb = iopool.tile([P, 1], f32)
                nc.sync.dma_start(
                    out=scale_sb, in_=scale.to_broadcast((P, 1))
                )
                wc0 = wpool.tile([P, KT, NT], f16)
                wc1 = wpool.tile([P, KT, NT], f16)
                for lo, hi in ((0, 4), (4, 16), (16, 32)):
                    nc.sync.dma_start(
                        out=wc0[:, lo:hi, :], in_=wt0[0, :, lo:hi, :]
                    )
                    nc.sync.dma_start(
                        out=wc1[:, lo:hi, :], in_=wt0[1, :, lo:hi, :]
                    )
                for t in range(1, TT):
                    nc.sync.dma_start(out=xt[:, t], in_=xt_d[:, t])
                bias_sb = iopool.tile([P, OUT_F], f32)
                nc.sync.dma_start(out=bias_sb, in_=bias)
                return xt, scale_sb, bias_sb, (wc0, wc1)

            def warmup():
                # Zero tile for PE warm-up matmuls (issued by _body into the
                # first accumulation tile, before its real j-loop).
                warm = iopool.tile([P, NT], f16)
                nc.vector.memset(warm, 0.0)
                return warm

            if FULL_REP:
                for rep in range(reps):
                    xt, scale_sb, bias_sb, p0w = prologue()
                    warm = warmup() if rep == 0 else None
                    _body(nc, wpool, opool, pacc_pool, wt, out, xt,
                          scale_sb, bias_sb, warm, p0w)
            else:
                xt, scale_sb, bias_sb, p0w = prologue()
                warm = warmup()
                for rep in range(reps):
                    _body(nc, wpool, opool, pacc_pool, wt, out, xt,
                          scale_sb, bias_sb, warm, p0w)
                    warm, p0w = None, None
    if DEDUPE_LDW:
        _dedupe_ldweights(nc)
    return nc


def _body(nc, wpool, opool, pacc_pool, wt, out, xt, scale_sb, bias_sb,
          warm=None, p0w=None):
    import concourse.mybir as mybir
    f16 = mybir.dt.float16
    f32 = mybir.dt.float32
    for pair in range(PAIRS):
        of0, of1 = 2 * pair, 2 * pair + 1
        if pair == 0 and p0w is not None:
            # fp16 copies already streaming on the sync queue (prologue)
            wc0, wc1 = p0w
        else:
            wc0 = wpool.tile([P, KT, NT], f16)
            wc1 = wpool.tile([P, KT, NT], f16)
            nc.gpsimd.dma_start(out=wc0, in_=wt[of0])  # int8 -> fp16 cast
            nc.gpsimd.dma_start(out=wc1, in_=wt[of1])
        for t in range(TT):
            acc = pacc_pool.tile([P, 2, NT], f32)  # 2 PSUM banks
            if warm is not None and pair == 0 and t == 0:
                # ~20 zero matmuls with no DMA dependency: keeps the PE busy
                # (HAM clock-gate open) during the initial DMA gate. The
                # real j=0 matmul below re-clears the bank (start=True).
                for _ in range(20):
                    nc.tensor.matmul(
                        acc[:, 0, :], warm[:, 0:P], warm,
                        start=True, stop=True,
                    )
            for j in range(KT):
                lhsT = xt[:, t, j, :]
                first, last = (j == 0), (j == KT - 1)
                nc.tensor.matmul(
                    acc[:, 0, :], lhsT, wc0[:, j, :], start=first, stop=last
                )
                nc.tensor.matmul(
                    acc[:, 1, :], lhsT, wc1[:, j, :], start=first, stop=last
                )
            osb = opool.tile([P, 2, NT], f32)
            if pair == PAIRS - 1 and t == TT - 1:
                # last group: per-bank epilogue so the first half's store
                # overlaps the second half's scale/bias (shorter drain tail)
                for b in range(2):
                    nc.scalar.mul(osb[:, b, :], acc[:, b, :], scale_sb[:, :])
                    nc.vector.tensor_add(
                        osb[:, b, :], osb[:, b, :],
                        bias_sb[:, (of0 + b) * NT:(of0 + b + 1) * NT],
                    )
                    nc.sync.dma_start(
                        out=out[t * P:(t + 1) * P,
                                (of0 + b) * NT:(of0 + b + 1) * NT],
                        in_=osb[:, b, :],
                    )
            else:
                nc.scalar.mul(osb, acc, scale_sb[:, :])
                nc.vector.tensor_add(
                    osb, osb, bias_sb[:, of0 * NT:(of1 + 1) * NT]
                )
                nc.sync.dma_start(
                    out=out[t * P:(t + 1) * P, of0 * NT:(of1 + 1) * NT],
                    in_=osb,
                )


def prep_inputs(input, qweight, weight_scale, bias_param):
    """Host-side shard/repack. Returns per-core in_maps."""
    X = np.asarray(input, dtype=np.float32).reshape(TOK, IN_F)
    # int8 container for the int8-valued weights; the device DMA casts to fp16
    # (exact for integers in [-127,127]).
    q8 = np.asarray(qweight).astype(np.int8)
    # w_packed[of, p, j, n] = qweight[of*NT + n, j*P + p]
    wp = np.ascontiguousarray(
        q8.reshape(OF_CHUNKS, NT, KT, P).transpose(0, 3, 2, 1)
    )
    bias2 = np.ascontiguousarray(
        np.broadcast_to(
            np.asarray(bias_param, dtype=np.float32).reshape(1, OUT_F), (P, OUT_F)
        )
    )
    scale2 = np.ascontiguousarray(
        np.asarray(weight_scale, dtype=np.float32).reshape(1, 1)
    )
    # pair-0 chunks pre-cast to fp16 (exact) for the sync-queue fast path
    wt0 = np.ascontiguousarray(wp[0:2].astype(np.float16))
    in_maps = []
    for c in range(N_CORES):
        Xc = X[c * TOK_C:(c + 1) * TOK_C].astype(np.float16)
        # xt[p, t, j, c] = x[t*128+c, j*128+p]
        xt = np.ascontiguousarray(
            Xc.reshape(TT, P, KT, P).transpose(3, 0, 2, 1)
        )
        in_maps.append(
            {
                "xt": xt,
                "wt": wp,
                "wt0": wt0,
                "bias": bias2,
                "scale": scale2,
            }
        )
    return in_maps


def assemble_output(results):
    out = np.concatenate([results[c]["out"] for c in range(N_CORES)], axis=0)
    return np.ascontiguousarray(out.reshape(B, S, OUT_F).astype(np.float32))


def kernel(input, qweight, weight_scale, bias_param):
    from concourse.bass_utils import run_bass_kernel_spmd

    in_maps = prep_inputs(input, qweight, weight_scale, bias_param)
    nc = build_nc()
    res = run_bass_kernel_spmd(nc, in_maps, core_ids=list(range(N_CORES)))
    return assemble_output(res.results)
